# revision 13
# baseline (speedup 1.0000x reference)
# Self-contained Trainium2 Bass kernel for nn_GAT (GNN message passing).
"""GAT kernel v3.

Changes over v2 (baseline 2.19ms):
  - Table rows live at PHYSICAL row  m*sp + p*nt + t  (partition-major within
    a core's slice) so every SBUF->DRAM table/slice write is a long
    contiguous run per partition (128 descriptors per flush instead of
    128 per TILE).
  - Layer-0 table is built REDUNDANTLY on every core from the replicated
    full x (xTfull input) -> no layer-0 AllGather (each collective has a
    ~130-400us floor in this environment).
  - Gathers read 144B per edge (elem_size=72, elem_step=128) via
    dma_gather_narrow -- the %256 elem assert in bass is transpose-only in
    the ucode; only the row PITCH must be a multiple of 256B.
  - Edge index tiles are persistent in SBUF (loaded once, reused by all
    3 layers).

Layout (per core, SPMD across 8 cores):
  - Nodes sharded by graph id; within a core sorted by max(degA,degB) desc
    (degrees EXCLUDE self-loops). OLD local row l = t*128+p (SBUF layout);
    PHYSICAL table row r = m*sp + p*nt + t.  Sentinel = physical rows
    T0_ROWS-1 (A) / Npad-1 (B): h=0, a_src=-1e30.
  - Table row = [h(64) bf16 | a_src(4 f32 as 8 bf16 raw) | junk] 256B.
  - Edge slots (tile, partition, slot) padded per (group, A|B) to uniform L;
    padding idx -> sentinel rows.  Self-loop edges are handled locally.
"""

from contextlib import ExitStack

import numpy as np

import concourse.bass as bass
import concourse.tile as tile
from concourse import bacc
from concourse import mybir
from concourse.library_config import mlp as mlp_lib
from concourse.masks import make_identity

F32 = mybir.dt.float32
BF16 = mybir.dt.bfloat16
I16 = mybir.dt.int16
AF = mybir.ActivationFunctionType
OP = mybir.AluOpType
P = 128

HID = 16
HEADS = 4
F_HID = HID * HEADS  # 64
NEG_SLOPE = 0.2
NEG_BIG = -1.0e30


def dma_gather_narrow(
    gps, out_ap, in_ap, idxs_ap, num_idxs, num_idxs_reg, elem_size,
    elem_step, queue_num=0, single_packet=True,
):
    """nc.gpsimd.dma_gather (non-transpose, DRAM source) minus the
    `elem_size_bytes % 256 == 0` assert. The ucode only requires the table
    row PITCH (elem_step bytes) to be a multiple of 256; the per-index read
    length is free. Mirrors bass.BassGpSimd.dma_gather."""
    import concourse.ap_utils as ap_utils
    from concourse.bass import MemorySpace

    gps._assert_queue_num(queue_num)
    assert idxs_ap.dtype == mybir.dt.int16
    assert in_ap.space == MemorySpace.DRAM
    assert idxs_ap.space == MemorySpace.SBUF
    assert out_ap.space == MemorySpace.SBUF
    assert in_ap.dtype == out_ap.dtype
    assert ap_utils.ap_is_contiguous(in_ap.ap[1:])
    assert ap_utils.ap_is_contiguous(out_ap.ap[1:])
    assert ap_utils.ap_is_contiguous(idxs_ap.ap[1:])
    assert in_ap.ap[-1][1] == out_ap.ap[-1][1] == elem_size
    assert out_ap.ap[0][1] * out_ap.ap[1][1] >= num_idxs
    assert in_ap.ap[0][0] == elem_step
    stride_bytes = elem_step * mybir.dt.size(in_ap.dtype)
    stride_bytes_256 = stride_bytes // 256
    assert stride_bytes % 256 == 0 and stride_bytes_256 < 256

    _in_ap = gps.lower_ap_dma(in_ap, for_custom_bir_dma=True)
    _idxs_ap = gps.lower_ap(idxs_ap)
    _out_ap = gps.lower_ap(out_ap)
    return gps.add_instruction(
        mybir.InstDMAGatherAnt(
            name=gps.bass.get_next_instruction_name(),
            ins=[*_in_ap, _idxs_ap, gps.lower_val_access(gps.to_reg(num_idxs_reg))],
            outs=[_out_ap],
            transpose=False,
            num_idxs=num_idxs,
            elem_size=elem_size,
            stride_bytes_256=stride_bytes_256,
            gen_mode=0,
            single_packet=single_packet,
            queue_num=queue_num,
            sbuf_tokens_per_rank=0,
            sbuf_free_dim_per_rank=0,
            sbuf_free_dim_pad_per_rank=0,
            sbuf_byte_offset=0,
        )
    )


def build_plan2(x, edge_index, batch, ng=128, ncores=8, ccap=64, tigmax=8):
    x = np.asarray(x, np.float32)
    ei = np.asarray(edge_index, np.int64)
    batch = np.asarray(batch, np.int64)
    N, f_in = x.shape
    g_per_core = ng // ncores

    src = ei[0]
    dst = ei[1]

    node_core = batch // g_per_core
    counts = np.bincount(node_core, minlength=ncores)
    slice_pad = int(np.ceil((counts.max() + 1) / P) * P)  # +1: sentinel room
    n_tiles = slice_pad // P
    Npad = ncores * slice_pad
    T0C = ncores // 2
    T0_ROWS = T0C * slice_pad
    assert T0_ROWS < 32768 and (Npad - T0_ROWS) < 32768, (T0_ROWS, Npad)
    core_start = np.concatenate([[0], np.cumsum(counts)])
    SENT_A = T0_ROWS - 1
    SENT_B = Npad - T0_ROWS - 1  # relative to T0

    src_core = node_core[src]
    isA = src_core < T0C
    degA = np.bincount(dst, weights=isA.astype(np.float64), minlength=N).astype(np.int64)
    degB = np.bincount(dst, weights=(~isA).astype(np.float64), minlength=N).astype(np.int64)

    # OLD local row l = t*128 + p (SBUF layout); PHYS row r = m*sp + p*nt + t
    glob_row = np.empty(N, np.int64)       # node -> OLD global row
    row_node = np.full(Npad, -1, np.int64)  # OLD global row -> node
    import os as _osp

    sortkey = _osp.environ.get("GAT_SORTKEY", "ilv")
    for m in range(ncores):
        lo, hi = core_start[m], core_start[m + 1]
        nodes = np.arange(lo, hi)
        if sortkey == "ilv":
            key = np.maximum(degA[nodes] * 2, degB[nodes] * 2 + 1)
        else:
            key = (
                np.maximum(degA[nodes], degB[nodes]) * 100000
                + degA[nodes] + degB[nodes]
            )
        nodes = nodes[np.argsort(-key, kind="stable")]
        glob_row[nodes] = m * slice_pad + np.arange(len(nodes))
        row_node[m * slice_pad + np.arange(len(nodes))] = nodes

    # old local l -> phys local l'
    l_old = np.arange(slice_pad)
    old2phys = (l_old % P) * n_tiles + (l_old // P)

    tile_degA = np.zeros((ncores, n_tiles), np.int64)
    tile_degB = np.zeros((ncores, n_tiles), np.int64)
    for m in range(ncores):
        rows = np.arange(counts[m])
        nodes = row_node[m * slice_pad + rows]
        t = rows // P
        np.maximum.at(tile_degA[m], t, degA[nodes])
        np.maximum.at(tile_degB[m], t, degB[nodes])
    gmaxA = tile_degA.max(axis=0)
    gmaxB = tile_degB.max(axis=0)

    groups = []
    t = 0
    while t < n_tiles:
        tig = 1
        while (
            t + tig < n_tiles
            and tig < tigmax
            and (tig + 1)
            * (max(gmaxA[t : t + tig + 1]) + max(gmaxB[t : t + tig + 1]))
            <= ccap
        ):
            tig += 1
        L0 = int(max(gmaxA[t : t + tig]))
        L1 = int(max(gmaxB[t : t + tig]))
        groups.append(dict(base=t, tig=tig, L0=L0, L1=L1))
        t += tig

    offA = offB = 0
    for g in groups:
        cA, cB = g["tig"] * g["L0"], g["tig"] * g["L1"]
        g["offA"], g["wA"] = offA, cA * 8
        g["offB"], g["wB"] = offB, cB * 8
        offA += g["wA"]
        offB += g["wB"]
    WA, WB = max(offA, 8), max(offB, 8)
    CM = sum(g["tig"] * (g["L0"] + g["L1"]) for g in groups)

    # per-core edge slot assignment (dst side: OLD layout tile/partition)
    dst_owner = node_core[dst]
    loc_row = glob_row[dst] - dst_owner * slice_pad
    # source PHYS row for gather indices
    src_m = glob_row[src] // slice_pad
    src_l = glob_row[src] % slice_pad
    src_row = src_m * slice_pad + old2phys[src_l]

    order = np.lexsort((src_row, ~isA, loc_row, dst_owner))
    so_owner = dst_owner[order]
    so_loc = loc_row[order]
    so_isA = isA[order]
    so_srcrow = src_row[order]
    key = so_owner * (Npad * 2) + so_loc * 2 + (~so_isA).astype(np.int64)
    newrun = np.concatenate([[True], key[1:] != key[:-1]])
    run_start = np.flatnonzero(newrun)
    slot = np.arange(len(key)) - run_start[np.cumsum(newrun) - 1]

    tile_of = so_loc // P
    part_of = so_loc % P
    group_of = np.zeros(n_tiles, np.int64)
    for gi, g in enumerate(groups):
        group_of[g["base"] : g["base"] + g["tig"]] = gi

    idxA_cat, idxB_cat = [], []
    for m in range(ncores):
        emask = so_owner == m
        et = tile_of[emask]
        ep = part_of[emask]
        eA = so_isA[emask]
        esrc = so_srcrow[emask]
        eslot = slot[emask]
        eg = group_of[et]

        iA_full = np.full((16, WA), SENT_A, np.int16)
        iB_full = np.full((16, WB), SENT_B, np.int16)
        for gi, g in enumerate(groups):
            tig, l0, l1 = g["tig"], g["L0"], g["L1"]
            gselA = (eg == gi) & eA
            gselB = (eg == gi) & ~eA
            tt = et - g["base"]
            if l0:
                cidx = tt[gselA] * l0 + eslot[gselA]
                q = cidx * P + ep[gselA]
                iA_full[q % 16, g["offA"] + q // 16] = esrc[gselA].astype(np.int16)
                assert esrc[gselA].max(initial=0) < T0_ROWS
            if l1:
                cidx = tt[gselB] * l1 + eslot[gselB]
                q = cidx * P + ep[gselB]
                iB_full[q % 16, g["offB"] + q // 16] = (
                    esrc[gselB] - T0_ROWS
                ).astype(np.int16)
                assert (esrc[gselB] - T0_ROWS).max(initial=0) < Npad - T0_ROWS
        idxA_cat.append(np.tile(iA_full, (8, 1)))
        idxB_cat.append(np.tile(iB_full, (8, 1)))

    import ml_dtypes

    # per-core xT slice [f_in, slice_pad] (OLD column order), bf16
    xT = []
    for m in range(ncores):
        xs = np.zeros((slice_pad, f_in), np.float32)
        nodes = row_node[m * slice_pad : m * slice_pad + counts[m]]
        xs[: counts[m]] = x[nodes]
        xT.append(np.ascontiguousarray(xs.T).astype(ml_dtypes.bfloat16))

    # replicated full xT [f_in, Npad] (OLD global column order), bf16
    xs_full = np.zeros((Npad, f_in), np.float32)
    valid = row_node >= 0
    xs_full[valid] = x[row_node[valid]]
    xTfull = np.ascontiguousarray(xs_full.T).astype(ml_dtypes.bfloat16)

    # selectors [128, n_tiles*g_per_core] per core (OLD layout)
    sel = []
    for m in range(ncores):
        s = np.zeros((P, n_tiles, g_per_core), np.float32)
        rows = np.arange(counts[m])
        nodes = row_node[m * slice_pad + rows]
        s[rows % P, rows // P, batch[nodes] - m * g_per_core] = 1.0
        sel.append(s.reshape(P, n_tiles * g_per_core))

    struct = dict(
        slice_pad=slice_pad,
        n_tiles=n_tiles,
        Npad=Npad,
        T0_ROWS=T0_ROWS,
        groups=groups,
        WA=WA,
        WB=WB,
        CM=CM,
        f_in=f_in,
        g_per_core=g_per_core,
        ncls=None,
        ncores=ncores,
        ng=ng,
    )
    glob = dict(glob_row=glob_row, row_node=row_node, counts=counts,
                old2phys=old2phys)
    percore = dict(idxA=idxA_cat, idxB=idxB_cat, xT=xT, sel=sel, xTfull=xTfull)
    return struct, percore, glob


def augment_weights(W, a_s, a_d):
    W = np.asarray(W, np.float32)
    a_s = np.asarray(a_s, np.float32)
    a_d = np.asarray(a_d, np.float32)
    As = np.zeros((F_HID, HEADS), np.float32)
    Ad = np.zeros((F_HID, HEADS), np.float32)
    for h in range(HEADS):
        As[h * HID : (h + 1) * HID, h] = a_s[h]
        Ad[h * HID : (h + 1) * HID, h] = a_d[h]
    return np.concatenate([W, W @ As, W @ Ad], axis=1).astype(np.float32)


def make_inmaps2(inputs, struct, percore, layers=3):
    ncores = struct["ncores"]
    ws = [
        augment_weights(inputs[f"W{l}"], inputs[f"as{l}"], inputs[f"ad{l}"])
        for l in range(layers)
    ]
    biases = np.concatenate(
        [np.asarray(inputs[f"b{l}"], np.float32) for l in range(layers)]
    )
    bias_rep = np.tile(biases[None, :], (P, 1))
    wr = np.asarray(inputs["Wr"], np.float32)
    br_rep = np.tile(np.asarray(inputs["br"], np.float32)[None, :], (P, 1))
    in_maps = []
    for m in range(ncores):
        im = dict(
            xT=percore["xT"][m],
            xTfull=percore["xTfull"],
            idxA=percore["idxA"][m],
            idxB=percore["idxB"][m],
            sel=percore["sel"][m],
            biases=bias_rep,
            wr=wr,
            br=br_rep,
        )
        for l in range(layers):
            im[f"w{l}aug"] = ws[l]
        in_maps.append(im)
    return in_maps


def numpy_model2(inputs, struct, percore, glob, layers=3):
    """Numpy re-implementation of the v3 device algorithm."""
    sp = struct["slice_pad"]
    Npad = struct["Npad"]
    T0 = struct["T0_ROWS"]
    ncores = struct["ncores"]
    gpc = struct["g_per_core"]
    nt = struct["n_tiles"]
    o2p = glob["old2phys"]
    ncls = np.asarray(inputs["Wr"]).shape[1]

    ws = [
        augment_weights(inputs[f"W{l}"], inputs[f"as{l}"], inputs[f"ad{l}"])
        for l in range(layers)
    ]
    biases = [np.asarray(inputs[f"b{l}"], np.float32) for l in range(layers)]

    import ml_dtypes

    acts = [None] * ncores
    out_logits = np.zeros((struct["ng"], ncls), np.float32)

    for layer in range(layers):
        table = np.zeros((Npad, 72), np.float32)  # PHYS rows
        for m in range(ncores):
            if layer == 0:
                a = percore["xT"][m].T.astype(np.float32)
            else:
                a = acts[m]
            vals = (a.astype(ml_dtypes.bfloat16).astype(np.float32)
                    @ ws[layer].astype(ml_dtypes.bfloat16).astype(np.float32))
            table[m * sp + o2p, :72] = vals
        # bf16 rounding of h columns (table stored bf16)
        table[:, :64] = table[:, :64].astype(ml_dtypes.bfloat16).astype(np.float32)
        table[T0 - 1, :64] = 0.0
        table[T0 - 1, 64:68] = NEG_BIG
        table[Npad - 1, :64] = 0.0
        table[Npad - 1, 64:68] = NEG_BIG
        for m in range(ncores):
            out = np.zeros((sp, F_HID), np.float32)
            lvals = table[m * sp + o2p, :]  # OLD-layout view of own rows
            asd = lvals[:, 64:72]
            l_self = asd[:, 0:4] + asd[:, 4:8]
            l_self = np.where(l_self >= 0, l_self, NEG_SLOPE * l_self)
            for g in struct["groups"]:
                tig, l0, l1 = g["tig"], g["L0"], g["L1"]
                cA, cB = tig * l0, tig * l1
                C = cA + cB
                iw = percore["idxA"][m][:16, g["offA"] : g["offA"] + g["wA"]]
                iA = iw.T.reshape(-1)[: cA * P].astype(np.int64)
                iw = percore["idxB"][m][:16, g["offB"] : g["offB"] + g["wB"]]
                iB = iw.T.reshape(-1)[: cB * P].astype(np.int64)
                GA = table[:T0][iA].reshape(cA, P, 72).transpose(1, 0, 2)
                GB = table[T0:][iB].reshape(cB, P, 72).transpose(1, 0, 2)
                G = np.concatenate([GA, GB], axis=1)  # [P, C, 72]
                tt = np.concatenate(
                    [np.repeat(np.arange(tig), l0), np.repeat(np.arange(tig), l1)]
                )
                node_rows = (g["base"] + tt)[None, :] * P + np.arange(P)[:, None]
                a_d = lvals[:, 68:72][node_rows]  # [P,C,4]
                logits = G[:, :, 64:68] + a_d
                logits = np.where(logits >= 0, logits, NEG_SLOPE * logits).astype(
                    np.float32
                )
                tiles = g["base"] + np.arange(tig)
                ls_g = l_self[
                    tiles[None, :] * P + np.arange(P)[:, None]
                ]  # [P,tig,4]
                mx = np.full((P, tig, HEADS), NEG_BIG, np.float32)
                np.maximum.at(mx, (slice(None), tt), logits)
                mx = np.maximum(mx, ls_g)
                e = np.exp(logits - mx[:, tt])
                e_self = np.exp(ls_g - mx)
                s = e_self.copy()
                np.add.at(s, (slice(None), tt), e)
                alpha = e * (1.0 / s)[:, tt]
                alpha_self = e_self * (1.0 / s)
                msg = G[:, :, :64].reshape(P, C, HEADS, HID) * alpha[:, :, :, None]
                acc = np.zeros((P, tig, F_HID), np.float32)
                np.add.at(acc, (slice(None), tt), msg.reshape(P, C, F_HID))
                h_self = lvals[:, :64][
                    tiles[None, :] * P + np.arange(P)[:, None]
                ]  # [P,tig,64]
                acc += (
                    h_self.reshape(P, tig, HEADS, HID)
                    * alpha_self[:, :, :, None]
                ).reshape(P, tig, F_HID)
                for t in range(tig):
                    out[(g["base"] + t) * P + np.arange(P)] = acc[:, t]
            act = out + biases[layer][None, :]
            act = act * (1.0 / (1.0 + np.exp(-act)))
            acts[m] = act.astype(np.float32)

    for m in range(ncores):
        sel = percore["sel"][m].reshape(P, nt, gpc)
        a = acts[m].reshape(nt, P, F_HID)
        pooled = np.einsum("ptg,tpf->gf", sel, a)
        lg = pooled @ np.asarray(inputs["Wr"]) + np.asarray(inputs["br"])
        lg = np.maximum(lg, 0.0)
        mxv = lg.max(axis=1, keepdims=True)
        ls = lg - mxv - np.log(np.exp(lg - mxv).sum(axis=1, keepdims=True))
        out_logits[m * gpc : (m + 1) * gpc] = ls
    return out_logits


def build_gat2(S, n_cores=8):
    import os

    dbg_layers = int(os.environ.get("GAT_NLAYERS", "3"))
    dbg_edge = int(os.environ.get("GAT_EDGE", "1"))
    dbg_compute = int(os.environ.get("GAT_COMPUTE", "1"))
    dbg_ngroups = int(os.environ.get("GAT_NGROUPS", "999"))
    dbg_skipcoll = int(os.environ.get("GAT_SKIPCOLL", "0"))
    ELEM = int(os.environ.get("GAT_ELEM", "72"))     # bf16 elems gathered/row
    REPL0 = int(os.environ.get("GAT_REPL0", "0"))    # replicated layer-0 build
    GCH = int(os.environ.get("GAT_GCH", "8"))
    MAXFREE = int(os.environ.get("GAT_MAXFREE", "1"))
    KFLUSH = int(os.environ.get("GAT_KFLUSH", "10"))  # tiles per table flush
    sp = S["slice_pad"]
    nt = S["n_tiles"]
    Npad = S["Npad"]
    T0 = S["T0_ROWS"]
    groups = S["groups"]
    WA = S["WA"]
    WB = S["WB"]
    GPC = S["g_per_core"]
    NCLS = S["ncls"]
    FH = F_HID
    LAYERS = 3
    in_dims = [S["f_in"], FH, FH]

    nc = bacc.Bacc("TRN2", debug=False, num_devices=n_cores, num_swdge_queues=4)

    # ---------------- I/O ----------------
    xT_d = nc.dram_tensor("xT", [in_dims[0], sp], BF16, kind="ExternalInput")
    xTfull_d = nc.dram_tensor("xTfull", [in_dims[0], Npad], BF16,
                              kind="ExternalInput")
    idxA_d = nc.dram_tensor("idxA", [P, WA], I16, kind="ExternalInput")
    idxB_d = nc.dram_tensor("idxB", [P, WB], I16, kind="ExternalInput")
    sel_d = nc.dram_tensor("sel", [P, nt * GPC], F32, kind="ExternalInput")
    w_d = [
        nc.dram_tensor(f"w{l}aug", [in_dims[l], 72], F32, kind="ExternalInput")
        for l in range(LAYERS)
    ]
    bias_d = nc.dram_tensor("biases", [P, LAYERS * FH], F32, kind="ExternalInput")
    wr_d = nc.dram_tensor("wr", [FH, NCLS], F32, kind="ExternalInput")
    br_d = nc.dram_tensor("br", [P, NCLS], F32, kind="ExternalInput")
    out_d = nc.dram_tensor("out", [GPC, NCLS], F32, kind="ExternalOutput")

    table_d = [
        nc.dram_tensor(f"table{l}", [Npad, P], BF16, addr_space="Shared")
        for l in range(LAYERS)
    ]

    rg = [list(range(n_cores))]
    tigmax = max(g["tig"] for g in groups)
    # slices for layers 1.. (layer 0 table is built replicated, no AG)
    slice_d = [
        nc.dram_tensor(f"slice{l}", [sp, P], BF16) for l in range(LAYERS)
    ]

    with tile.TileContext(nc) as tc, ExitStack() as ctx:
        pers = ctx.enter_context(tc.tile_pool(name="pers", bufs=1))
        gpool = ctx.enter_context(
            tc.tile_pool(name="G", bufs=int(os.environ.get("GAT_GBUFS", "5")))
        )
        ltpool = ctx.enter_context(tc.tile_pool(name="lt", bufs=3))
        xpool = ctx.enter_context(tc.tile_pool(name="xchunk", bufs=4))
        stat = ctx.enter_context(tc.tile_pool(name="stat", bufs=3))
        opool = ctx.enter_context(tc.tile_pool(name="oacc", bufs=3))
        rowp = ctx.enter_context(tc.tile_pool(name="row", bufs=3))
        psum = ctx.enter_context(tc.tile_pool(name="psum", bufs=2, space="PSUM"))
        psumT = ctx.enter_context(tc.tile_pool(name="psumT", bufs=2, space="PSUM"))

        # ---- persistent SBUF ----
        sel_sb = pers.tile([P, nt * GPC], F32)
        w_sb = [
            pers.tile([in_dims[l], 72], F32, name=f"w{l}sb", tag=f"w{l}sb")
            for l in range(LAYERS)
        ]
        wb_sb = [
            pers.tile([in_dims[l], 72], BF16, name=f"wb{l}sb", tag=f"wb{l}sb")
            for l in range(LAYERS)
        ]
        bias_sb = pers.tile([P, LAYERS * FH], F32)
        wr_sb = pers.tile([FH, NCLS], F32)
        br_sb = pers.tile([P, NCLS], F32)
        idxA_sb = pers.tile([P, WA], I16)
        idxB_sb = pers.tile([P, WB], I16)
        # rows staged at 256B pitch so table/slice flushes are contiguous
        rows_sb = pers.tile([P, nt * P], BF16)    # [h(64)|a_src raw(8)|junk]
        asd_sb = pers.tile([P, nt * 8], F32)      # [a_src(4)|a_dst(4)] per tile
        lself_sb = pers.tile([P, nt * HEADS], F32)  # leaky(a_s+a_d) per tile
        out_sb = pers.tile([P, nt * FH], F32)
        actT_sb = pers.tile([FH, sp], BF16)
        xT_sb = pers.tile([in_dims[0], sp], BF16)
        nc.sync.dma_start(xT_sb[:], xT_d[:])
        ident = pers.tile([P, P], F32)

        nc.sync.dma_start(sel_sb[:], sel_d[:])
        for l in range(LAYERS):
            nc.sync.dma_start(w_sb[l][:], w_d[l][:])
        nc.sync.dma_start(bias_sb[:], bias_d[:])
        nc.sync.dma_start(wr_sb[:], wr_d[:])
        nc.sync.dma_start(br_sb[:], br_d[:])
        nc.sync.dma_start(idxA_sb[:], idxA_d[:])
        nc.sync.dma_start(idxB_sb[:], idxB_d[:])
        for l in range(LAYERS):
            nc.vector.tensor_copy(wb_sb[l][:], w_sb[l][:])
        make_identity(nc, ident[:])

        nc.gpsimd.load_library(mlp_lib)

        _regs = {}
        qrr = [0]

        def nreg(v):
            if v not in _regs:
                _regs[v] = nc.gpsimd.to_reg(v)
            return _regs[v]

        rows3 = rows_sb[:].rearrange("p (t f) -> p t f", f=P)
        asd3 = asd_sb[:].rearrange("p (t x) -> p t x", x=8)
        lself3 = lself_sb[:].rearrange("p (t h) -> p t h", h=HEADS)

        # sentinel row constant: h=0, a_src=-1e30 (raw f32 bits)
        sent_sb = pers.tile([1, 72], BF16, tag="sent")
        nc.vector.memset(sent_sb[:, :64], 0.0)
        nc.vector.memset(sent_sb[:, 64:72].bitcast(F32), NEG_BIG)

        # PHYS view of a table/slice: rows (p nt + t) -> [p, t, elem]
        def phys_rows(dram, lo_row, n_rows_tiles):
            return dram[lo_row : lo_row + P * n_rows_tiles, :].rearrange(
                "(p t) e -> p t e", t=n_rows_tiles
            )

        def build_table0_repl():
            """Every core writes the FULL layer-0 table from xTfull."""
            for m in range(n_cores):
                t = 0
                while t < nt:
                    k = min(KFLUSH, nt - t)
                    st = rowp.tile([P, KFLUSH * P], BF16, tag="t0stage")
                    for j in range(k):
                        xc = xpool.tile([in_dims[0], P], BF16, tag="xfc")
                        col = m * sp + (t + j) * P
                        nc.sync.dma_start(xc[:], xTfull_d[:, col : col + P])
                        pR = psum.tile([P, 72], F32)
                        nc.tensor.matmul(
                            pR[:], lhsT=xc[:], rhs=wb_sb[0][:], start=True, stop=True
                        )
                        sl = st[:].rearrange("p (j f) -> p j f", f=P)
                        nc.scalar.copy(sl[:, j, :64], pR[:, :64])
                        nc.vector.tensor_copy(
                            sl[:, j, 64:72].bitcast(F32), pR[:, 64:68]
                        )
                    # flush k tiles: phys rows m*sp + p*nt + (t..t+k)
                    nc.sync.dma_start(
                        phys_rows(table_d[0], m * sp, nt)[:, t : t + k, :],
                        st[:].rearrange("p (j f) -> p j f", f=P)[:, :k, :],
                    )
                    t += k
            # sentinel rows (phys T0-1 and Npad-1)
            nc.sync.dma_start(table_d[0][T0 - 1 : T0, :72], sent_sb[:])
            nc.sync.dma_start(table_d[0][Npad - 1 : Npad, :72], sent_sb[:])

        def build_slice_tile(layer, c, write_slice=True):
            """Matmul tile c of layer's rows into rows_sb + asd_sb (+ lself).
            For layer 0 lhsT comes from xT (local extras only)."""
            if layer == 0:
                lhsT_ap = xT_sb[:, c * P : (c + 1) * P]
            else:
                pT = psumT.tile([FH, P], F32)
                nc.tensor.transpose(
                    out=pT[:],
                    in_=out_sb[:, c * FH : (c + 1) * FH],
                    identity=ident[:],
                )
                nc.scalar.copy(actT_sb[:, c * P : (c + 1) * P], pT[:])
                lhsT_ap = actT_sb[:, c * P : (c + 1) * P]
            pR = psum.tile([P, 72], F32)
            nc.tensor.matmul(pR[:], lhsT=lhsT_ap, rhs=wb_sb[layer][:], start=True,
                             stop=True)
            nc.scalar.copy(rows3[:, c, :64], pR[:, :64])  # h -> bf16
            nc.vector.tensor_copy(rows3[:, c, 64:72].bitcast(F32), pR[:, 64:68])
            nc.vector.tensor_copy(asd3[:, c, :], pR[:, 64:72])
            # l_self = leaky(a_src + a_dst)
            ls = lself3[:, c, :]
            nc.vector.tensor_tensor(out=ls, in0=asd3[:, c, 0:4],
                                    in1=asd3[:, c, 4:8], op=OP.add)
            lst = stat.tile([P, HEADS], F32, tag="lst")
            nc.vector.tensor_scalar_mul(lst[:], ls, NEG_SLOPE)
            nc.vector.tensor_tensor(out=ls, in0=ls, in1=lst[:], op=OP.max)


        def flush_slice(layer, t0, t1):
            """Write rows_sb tiles [t0,t1) into slice_d[layer] (phys rows)."""
            nc.sync.dma_start(
                phys_rows(slice_d[layer], 0, nt)[:, t0:t1, :],
                rows3[:, t0:t1, :],
            )

        def do_allgather(layer):
            if dbg_skipcoll:
                return
            nc.gpsimd.collective_compute(
                "AllGather",
                mybir.AluOpType.bypass,
                replica_groups=rg,
                ins=[slice_d[layer].ap().opt()],
                outs=[table_d[layer].ap().opt()],
            )

        def edge_group(layer, g):
            tig, l0, l1 = g["tig"], g["L0"], g["L1"]
            cA, cB = tig * l0, tig * l1
            C = cA + cB
            base = g["base"]

            if C == 0:
                osl = out_sb[:, base * FH : (base + tig) * FH]
                nc.vector.tensor_copy(
                    osl.rearrange("p (t f) -> p t f", f=FH),
                    rows3[:, base : base + tig, :64],
                )
                return

            G = gpool.tile([P, C * ELEM], BF16, tag="G")
            G3 = G[:].rearrange("p (c f) -> p c f", f=ELEM)
            for c0all, ccn, itile, ioff, tdsl in (
                (0, cA, idxA_sb, g["offA"], table_d[layer][:T0, :ELEM]),
                (cA, cB, idxB_sb, g["offB"], table_d[layer][T0:, :ELEM]),
            ):
                if ccn == 0:
                    continue
                for k in range(0, ccn, GCH):
                    kc = min(GCH, ccn - k)
                    dma_gather_narrow(
                        nc.gpsimd,
                        G3[:, c0all + k : c0all + k + kc, :],
                        tdsl,
                        itile[:, (ioff + k * 8) : (ioff + (k + kc) * 8)],
                        kc * P,
                        nreg(kc * P),
                        ELEM,
                        P,
                        queue_num=qrr[0] % 4,
                    )
                    qrr[0] += 1

            if not dbg_compute:
                nc.vector.tensor_copy(
                    out_sb[:, base * FH : (base + tig) * FH].rearrange(
                        "p (t f) -> p t f", f=64
                    ),
                    G3[:, :tig, :64],
                )
                return
            lt = ltpool.tile([P, C * HEADS], F32, tag="lt")
            lt3 = lt[:].rearrange("p (c h) -> p c h", h=HEADS)
            # logits = a_src[gathered] + a_dst[local]
            for c0, cc, L in ((0, cA, l0), (cA, cB, l1)):
                if cc == 0:
                    continue
                adview = (
                    asd3[:, base : base + tig, 4:8]
                    .unsqueeze(2)
                    .broadcast_to([P, tig, L, HEADS])
                )
                nc.vector.tensor_tensor(
                    out=lt3[:, c0 : c0 + cc, :].rearrange(
                        "p (t l) h -> p t l h", l=L
                    ),
                    in0=G3[:, c0 : c0 + cc, 64:72]
                    .bitcast(F32)
                    .rearrange("p (t l) h -> p t l h", l=L),
                    in1=adview,
                    op=OP.add,
                )
            # leaky relu
            lt2 = ltpool.tile([P, C * HEADS], F32, tag="lt2")
            nc.vector.tensor_scalar_mul(lt2[:], lt[:], NEG_SLOPE)
            nc.vector.tensor_tensor(out=lt[:], in0=lt[:], in1=lt2[:], op=OP.max)
            es = stat.tile([P, tigmax * HEADS], F32, tag="es")
            if MAXFREE:
                # logits are small (|l| < ~20): skip the max-subtraction pass
                nc.scalar.activation(lt[:], lt[:], AF.Exp)
                nc.scalar.activation(
                    es[:, : tig * HEADS],
                    lself_sb[:, base * HEADS : (base + tig) * HEADS],
                    AF.Exp,
                )
            else:
                # segment max over slots + self
                m1 = stat.tile([P, tigmax * HEADS], F32, tag="m1")
                if cA:
                    nc.vector.reduce_max(
                        m1[:].rearrange("p (t h) -> p t h", h=HEADS)[:, :tig, :],
                        lt3[:, :cA, :].rearrange("p (t l) h -> p t h l", l=l0),
                        axis=mybir.AxisListType.X,
                    )
                if cB:
                    m2 = stat.tile([P, tigmax * HEADS], F32, tag="m2")
                    nc.vector.reduce_max(
                        m2[:].rearrange("p (t h) -> p t h", h=HEADS)[:, :tig, :],
                        lt3[:, cA:, :].rearrange("p (t l) h -> p t h l", l=l1),
                        axis=mybir.AxisListType.X,
                    )
                    nc.vector.tensor_tensor(
                        out=m1[:, : tig * HEADS],
                        in0=m1[:, : tig * HEADS] if cA else m2[:, : tig * HEADS],
                        in1=m2[:, : tig * HEADS],
                        op=OP.max,
                    )
                nc.vector.tensor_tensor(
                    out=m1[:, : tig * HEADS],
                    in0=m1[:, : tig * HEADS],
                    in1=lself_sb[:, base * HEADS : (base + tig) * HEADS],
                    op=OP.max,
                )
                # e = exp(l - m)
                for c0, cc, L in ((0, cA, l0), (cA, cB, l1)):
                    if cc == 0:
                        continue
                    mview = (
                        m1[:, : tig * HEADS]
                        .rearrange("p (t h) -> p t h", h=HEADS)
                        .unsqueeze(2)
                        .broadcast_to([P, tig, L, HEADS])
                    )
                    ltv = lt3[:, c0 : c0 + cc, :].rearrange(
                        "p (t l) h -> p t l h", l=L
                    )
                    nc.vector.tensor_tensor(
                        out=ltv, in0=ltv, in1=mview, op=OP.subtract
                    )
                nc.scalar.activation(lt[:], lt[:], AF.Exp)
                # e_self = exp(l_self - m)
                nc.vector.tensor_tensor(
                    out=es[:, : tig * HEADS],
                    in0=lself_sb[:, base * HEADS : (base + tig) * HEADS],
                    in1=m1[:, : tig * HEADS],
                    op=OP.subtract,
                )
                nc.scalar.activation(
                    es[:, : tig * HEADS], es[:, : tig * HEADS], AF.Exp
                )
            # s = sum(e) + e_self ; r = 1/s
            s1 = stat.tile([P, tigmax * HEADS], F32, tag="s1")
            if cA:
                nc.vector.reduce_sum(
                    s1[:].rearrange("p (t h) -> p t h", h=HEADS)[:, :tig, :],
                    lt3[:, :cA, :].rearrange("p (t l) h -> p t h l", l=l0),
                    axis=mybir.AxisListType.X,
                )
            if cB:
                s2 = stat.tile([P, tigmax * HEADS], F32, tag="s2")
                nc.vector.reduce_sum(
                    s2[:].rearrange("p (t h) -> p t h", h=HEADS)[:, :tig, :],
                    lt3[:, cA:, :].rearrange("p (t l) h -> p t h l", l=l1),
                    axis=mybir.AxisListType.X,
                )
                nc.vector.tensor_tensor(
                    out=s1[:, : tig * HEADS],
                    in0=s1[:, : tig * HEADS] if cA else s2[:, : tig * HEADS],
                    in1=s2[:, : tig * HEADS],
                    op=OP.add,
                )
            nc.vector.tensor_tensor(
                out=s1[:, : tig * HEADS],
                in0=s1[:, : tig * HEADS],
                in1=es[:, : tig * HEADS],
                op=OP.add,
            )
            nc.vector.reciprocal(s1[:, : tig * HEADS], s1[:, : tig * HEADS])
            # alpha = e * r ; alpha_self = e_self * r
            ab = ltpool.tile([P, C * HEADS], BF16, tag="ab")
            ab3 = ab[:].rearrange("p (c h) -> p c h", h=HEADS)
            for c0, cc, L in ((0, cA, l0), (cA, cB, l1)):
                if cc == 0:
                    continue
                rview = (
                    s1[:, : tig * HEADS]
                    .rearrange("p (t h) -> p t h", h=HEADS)
                    .unsqueeze(2)
                    .broadcast_to([P, tig, L, HEADS])
                )
                ltv = lt3[:, c0 : c0 + cc, :].rearrange("p (t l) h -> p t l h", l=L)
                abv = ab3[:, c0 : c0 + cc, :].rearrange("p (t l) h -> p t l h", l=L)
                nc.vector.tensor_tensor(out=abv, in0=ltv, in1=rview, op=OP.mult)
            nc.vector.tensor_tensor(
                out=es[:, : tig * HEADS],
                in0=es[:, : tig * HEADS],
                in1=s1[:, : tig * HEADS],
                op=OP.mult,
            )
            # msg = h * alpha (in place on G, bf16), 4D broadcast over HID
            gv = G3[:, :C, :64].rearrange("p c (h d) -> p c h d", d=HID)
            av = (
                ab3[:, :C, :]
                .unsqueeze(3)
                .broadcast_to([P, C, HEADS, HID])
            )
            nc.vector.tensor_tensor(out=gv, in0=gv, in1=av, op=OP.mult)
            # out = segment sum of messages (+ self)
            oA = opool.tile([P, tigmax * FH], F32, tag="oA")
            oB = opool.tile([P, tigmax * FH], F32, tag="oB")
            if cA:
                nc.vector.reduce_sum(
                    oA[:].rearrange("p (t f) -> p t f", f=FH)[:, :tig, :],
                    G3[:, :cA, :64].rearrange("p (t l) f -> p t f l", l=l0),
                    axis=mybir.AxisListType.X,
                )
            if cB:
                nc.vector.reduce_sum(
                    oB[:].rearrange("p (t f) -> p t f", f=FH)[:, :tig, :],
                    G3[:, cA:, :64].rearrange("p (t l) f -> p t f l", l=l1),
                    axis=mybir.AxisListType.X,
                )
            osl = out_sb[:, base * FH : (base + tig) * FH]
            if cA and cB:
                nc.vector.tensor_tensor(
                    out=osl, in0=oA[:, : tig * FH], in1=oB[:, : tig * FH], op=OP.add
                )
            else:
                nc.vector.tensor_copy(
                    osl, (oA if cA else oB)[:, : tig * FH]
                )
            # self message: osl += h_local * alpha_self
            stmp = opool.tile([P, tigmax * FH], F32, tag="stmp")
            nc.vector.tensor_tensor(
                out=stmp[:, : tig * FH].rearrange(
                    "p (t h d) -> p t h d", h=HEADS, d=HID
                ),
                in0=rows3[:, base : base + tig, :64].rearrange(
                    "p t (h d) -> p t h d", d=HID
                ),
                in1=es[:, : tig * HEADS]
                .rearrange("p (t h) -> p t h", h=HEADS)
                .unsqueeze(3)
                .broadcast_to([P, tig, HEADS, HID]),
                op=OP.mult,
            )
            nc.vector.tensor_tensor(
                out=osl, in0=osl, in1=stmp[:, : tig * FH], op=OP.add
            )
            # bias + silu
            blg = (
                bias_sb[:, layer * FH : (layer + 1) * FH]
                .unsqueeze(1)
                .broadcast_to([P, tig, FH])
            )
            nc.vector.tensor_tensor(
                out=osl.rearrange("p (t f) -> p t f", f=FH),
                in0=osl.rearrange("p (t f) -> p t f", f=FH),
                in1=blg,
                op=OP.add,
            )
            sgg = stat.tile([P, tigmax * FH], F32, tag="sgg")
            nc.scalar.activation(sgg[:, : tig * FH], osl, AF.Sigmoid)
            nc.vector.tensor_tensor(
                out=osl, in0=osl, in1=sgg[:, : tig * FH], op=OP.mult
            )

        # ============ layer 0 table + local extras ============
        if REPL0:
            build_table0_repl()
            for c in range(nt):
                build_slice_tile(0, c, write_slice=False)
        else:
            for c in range(nt):
                build_slice_tile(0, c)
            flush_slice(0, 0, nt)
            # sentinel at phys slice row sp-1 (overwrites junk row)
            nc.sync.dma_start(slice_d[0][sp - 1 : sp, :72], sent_sb[:])
            do_allgather(0)

        # ============ layers ============
        pP = psum.tile([GPC, FH], F32, tag="pool", bufs=1)
        pool_done = 0  # tiles accumulated into the pooling PSUM so far

        def pool_tiles(hi):
            nonlocal pool_done
            for t in range(pool_done, hi):
                nc.tensor.matmul(
                    pP[:],
                    lhsT=sel_sb[:, t * GPC : (t + 1) * GPC],
                    rhs=out_sb[:, t * FH : (t + 1) * FH],
                    start=(t == 0),
                    stop=(t == nt - 1),
                )
            pool_done = hi

        for layer in range(dbg_layers):
            built = 0
            flushed = 0
            last = layer == dbg_layers - 1
            for gi, g in enumerate(groups[:dbg_ngroups] if dbg_edge else []):
                edge_group(layer, g)
                nxt = layer + 1
                if nxt < dbg_layers:
                    for c in range(built, g["base"] + g["tig"]):
                        build_slice_tile(nxt, c)
                    built = g["base"] + g["tig"]
                    if built - flushed >= KFLUSH:
                        flush_slice(nxt, flushed, built)
                        flushed = built
                elif last and dbg_edge and dbg_ngroups >= len(groups):
                    pool_tiles(g["base"] + g["tig"])
            nxt = layer + 1
            if nxt < dbg_layers:
                for c in range(built, nt):
                    build_slice_tile(nxt, c)
                flush_slice(nxt, flushed, nt)
                # sentinel at phys slice row sp-1 (overwrites junk row)
                nc.sync.dma_start(slice_d[nxt][sp - 1 : sp, :72], sent_sb[:])
                do_allgather(nxt)

        # ============ pooling + classifier ============
        pool_tiles(nt)
        pooled = rowp.tile([GPC, FH], F32, tag="pooled")
        nc.vector.tensor_copy(pooled[:], pP[:])
        pTpsum = psumT.tile([FH, GPC], F32, tag="poolT", bufs=1)
        nc.tensor.transpose(out=pTpsum[:], in_=pooled[:], identity=ident[:GPC, :GPC])
        pooledT = rowp.tile([FH, GPC], F32, tag="pooledT")
        nc.vector.tensor_copy(pooledT[:], pTpsum[:])
        lgP = psum.tile([GPC, NCLS], F32, tag="lg", bufs=1)
        nc.tensor.matmul(lgP[:], lhsT=pooledT[:], rhs=wr_sb[:], start=True, stop=True)
        lg = rowp.tile([GPC, NCLS], F32, tag="lgs")
        nc.vector.tensor_tensor(out=lg[:], in0=lgP[:], in1=br_sb[:GPC, :], op=OP.add)
        nc.scalar.activation(lg[:], lg[:], AF.Relu)
        mx = stat.tile([GPC, 1], F32, tag="mx")
        nc.vector.reduce_max(mx[:], lg[:], axis=mybir.AxisListType.X)
        nc.vector.tensor_tensor(
            out=lg[:], in0=lg[:], in1=mx[:].broadcast_to([GPC, NCLS]), op=OP.subtract
        )
        ex = rowp.tile([GPC, NCLS], F32, tag="ex")
        nc.scalar.activation(ex[:], lg[:], AF.Exp)
        sm = stat.tile([GPC, 1], F32, tag="sm")
        nc.vector.reduce_sum(sm[:], ex[:], axis=mybir.AxisListType.X)
        lnm = stat.tile([GPC, 1], F32, tag="lnm")
        nc.scalar.activation(lnm[:], sm[:], AF.Ln)
        nc.vector.tensor_tensor(
            out=lg[:], in0=lg[:], in1=lnm[:].broadcast_to([GPC, NCLS]), op=OP.subtract
        )
        nc.sync.dma_start(out_d[:], lg[:])

    nc.compile()
    return nc


# ======== runner ========
"""kernel(**inputs) entry point: shard, build, run on 8 cores, gather."""

import os as _os

_NCORES = 8
_NG = 128


def kernel(**inputs) -> "np.ndarray":
    x = np.asarray(inputs["x"], np.float32)
    ei = np.asarray(inputs["edge_index"])
    batch = np.asarray(inputs["batch"])

    struct, percore, glob = build_plan2(
        x, ei, batch, ng=_NG, ncores=_NCORES,
        ccap=int(_os.environ.get("GAT_CCAP", "64")),
    )
    struct["ncls"] = int(np.asarray(inputs["Wr"]).shape[1])

    nc = build_gat2(struct, n_cores=_NCORES)
    in_maps = make_inmaps2(inputs, struct, percore)

    from concourse.bass_utils import run_bass_kernel_spmd

    trace = _os.environ.get("GAT_TRACE", "0") == "1"
    res = run_bass_kernel_spmd(
        nc,
        in_maps,
        core_ids=list(range(_NCORES)),
        trace=trace,
    )
    if res.exec_time_ns is not None:
        print(f"HW exec time: {res.exec_time_ns} ns", flush=True)

    out = np.concatenate([res.results[m]["out"] for m in range(_NCORES)], axis=0)
    return out.astype(np.float32)


# revision 14
# speedup vs baseline: 1.0410x; 1.0410x over previous
# Self-contained Trainium2 Bass kernel for nn_GAT (GNN message passing).
"""GAT kernel v3.

Changes over v2 (baseline 2.19ms):
  - Table rows live at PHYSICAL row  m*sp + p*nt + t  (partition-major within
    a core's slice) so every SBUF->DRAM table/slice write is a long
    contiguous run per partition (128 descriptors per flush instead of
    128 per TILE).
  - Layer-0 table is built REDUNDANTLY on every core from the replicated
    full x (xTfull input) -> no layer-0 AllGather (each collective has a
    ~130-400us floor in this environment).
  - Gathers read 144B per edge (elem_size=72, elem_step=128) via
    dma_gather_narrow -- the %256 elem assert in bass is transpose-only in
    the ucode; only the row PITCH must be a multiple of 256B.
  - Edge index tiles are persistent in SBUF (loaded once, reused by all
    3 layers).

Layout (per core, SPMD across 8 cores):
  - Nodes sharded by graph id; within a core sorted by max(degA,degB) desc
    (degrees EXCLUDE self-loops). OLD local row l = t*128+p (SBUF layout);
    PHYSICAL table row r = m*sp + p*nt + t.  Sentinel = physical rows
    T0_ROWS-1 (A) / Npad-1 (B): h=0, a_src=-1e30.
  - Table row = [h(64) bf16 | a_src(4 f32 as 8 bf16 raw) | junk] 256B.
  - Edge slots (tile, partition, slot) padded per (group, A|B) to uniform L;
    padding idx -> sentinel rows.  Self-loop edges are handled locally.
"""

from contextlib import ExitStack

import numpy as np

import concourse.bass as bass
import concourse.tile as tile
from concourse import bacc
from concourse import mybir
from concourse.library_config import mlp as mlp_lib
from concourse.masks import make_identity

F32 = mybir.dt.float32
BF16 = mybir.dt.bfloat16
I16 = mybir.dt.int16
AF = mybir.ActivationFunctionType
OP = mybir.AluOpType
P = 128

HID = 16
HEADS = 4
F_HID = HID * HEADS  # 64
NEG_SLOPE = 0.2
NEG_BIG = -1.0e30


def dma_gather_narrow(
    gps, out_ap, in_ap, idxs_ap, num_idxs, num_idxs_reg, elem_size,
    elem_step, queue_num=0, single_packet=True,
):
    """nc.gpsimd.dma_gather (non-transpose, DRAM source) minus the
    `elem_size_bytes % 256 == 0` assert. The ucode only requires the table
    row PITCH (elem_step bytes) to be a multiple of 256; the per-index read
    length is free. Mirrors bass.BassGpSimd.dma_gather."""
    import concourse.ap_utils as ap_utils
    from concourse.bass import MemorySpace

    gps._assert_queue_num(queue_num)
    assert idxs_ap.dtype == mybir.dt.int16
    assert in_ap.space == MemorySpace.DRAM
    assert idxs_ap.space == MemorySpace.SBUF
    assert out_ap.space == MemorySpace.SBUF
    assert in_ap.dtype == out_ap.dtype
    assert ap_utils.ap_is_contiguous(in_ap.ap[1:])
    assert ap_utils.ap_is_contiguous(out_ap.ap[1:])
    assert ap_utils.ap_is_contiguous(idxs_ap.ap[1:])
    assert in_ap.ap[-1][1] == out_ap.ap[-1][1] == elem_size
    assert out_ap.ap[0][1] * out_ap.ap[1][1] >= num_idxs
    assert in_ap.ap[0][0] == elem_step
    stride_bytes = elem_step * mybir.dt.size(in_ap.dtype)
    stride_bytes_256 = stride_bytes // 256
    assert stride_bytes % 256 == 0 and stride_bytes_256 < 256

    _in_ap = gps.lower_ap_dma(in_ap, for_custom_bir_dma=True)
    _idxs_ap = gps.lower_ap(idxs_ap)
    _out_ap = gps.lower_ap(out_ap)
    return gps.add_instruction(
        mybir.InstDMAGatherAnt(
            name=gps.bass.get_next_instruction_name(),
            ins=[*_in_ap, _idxs_ap, gps.lower_val_access(gps.to_reg(num_idxs_reg))],
            outs=[_out_ap],
            transpose=False,
            num_idxs=num_idxs,
            elem_size=elem_size,
            stride_bytes_256=stride_bytes_256,
            gen_mode=0,
            single_packet=single_packet,
            queue_num=queue_num,
            sbuf_tokens_per_rank=0,
            sbuf_free_dim_per_rank=0,
            sbuf_free_dim_pad_per_rank=0,
            sbuf_byte_offset=0,
        )
    )


def build_plan2(x, edge_index, batch, ng=128, ncores=8, ccap=64, tigmax=8):
    x = np.asarray(x, np.float32)
    ei = np.asarray(edge_index, np.int64)
    batch = np.asarray(batch, np.int64)
    N, f_in = x.shape
    g_per_core = ng // ncores

    src = ei[0]
    dst = ei[1]

    node_core = batch // g_per_core
    counts = np.bincount(node_core, minlength=ncores)
    slice_pad = int(np.ceil((counts.max() + 1) / P) * P)  # +1: sentinel room
    n_tiles = slice_pad // P
    Npad = ncores * slice_pad
    T0C = ncores // 2
    T0_ROWS = T0C * slice_pad
    assert T0_ROWS < 32768 and (Npad - T0_ROWS) < 32768, (T0_ROWS, Npad)
    core_start = np.concatenate([[0], np.cumsum(counts)])
    SENT_A = T0_ROWS - 1
    SENT_B = Npad - T0_ROWS - 1  # relative to T0

    src_core = node_core[src]
    isA = src_core < T0C
    degA = np.bincount(dst, weights=isA.astype(np.float64), minlength=N).astype(np.int64)
    degB = np.bincount(dst, weights=(~isA).astype(np.float64), minlength=N).astype(np.int64)

    # OLD local row l = t*128 + p (SBUF layout); PHYS row r = m*sp + p*nt + t
    glob_row = np.empty(N, np.int64)       # node -> OLD global row
    row_node = np.full(Npad, -1, np.int64)  # OLD global row -> node
    import os as _osp

    sortkey = _osp.environ.get("GAT_SORTKEY", "max")
    for m in range(ncores):
        lo, hi = core_start[m], core_start[m + 1]
        nodes = np.arange(lo, hi)
        if sortkey == "ilv":
            key = np.maximum(degA[nodes] * 2, degB[nodes] * 2 + 1)
        else:
            key = (
                np.maximum(degA[nodes], degB[nodes]) * 100000
                + degA[nodes] + degB[nodes]
            )
        nodes = nodes[np.argsort(-key, kind="stable")]
        glob_row[nodes] = m * slice_pad + np.arange(len(nodes))
        row_node[m * slice_pad + np.arange(len(nodes))] = nodes

    # old local l -> phys local l'
    l_old = np.arange(slice_pad)
    old2phys = (l_old % P) * n_tiles + (l_old // P)

    tile_degA = np.zeros((ncores, n_tiles), np.int64)
    tile_degB = np.zeros((ncores, n_tiles), np.int64)
    for m in range(ncores):
        rows = np.arange(counts[m])
        nodes = row_node[m * slice_pad + rows]
        t = rows // P
        np.maximum.at(tile_degA[m], t, degA[nodes])
        np.maximum.at(tile_degB[m], t, degB[nodes])
    gmaxA = tile_degA.max(axis=0)
    gmaxB = tile_degB.max(axis=0)

    groups = []
    t = 0
    while t < n_tiles:
        tig = 1
        while (
            t + tig < n_tiles
            and tig < tigmax
            and (tig + 1)
            * (max(gmaxA[t : t + tig + 1]) + max(gmaxB[t : t + tig + 1]))
            <= ccap
        ):
            tig += 1
        L0 = int(max(gmaxA[t : t + tig]))
        L1 = int(max(gmaxB[t : t + tig]))
        groups.append(dict(base=t, tig=tig, L0=L0, L1=L1))
        t += tig

    offA = offB = 0
    for g in groups:
        cA, cB = g["tig"] * g["L0"], g["tig"] * g["L1"]
        g["offA"], g["wA"] = offA, cA * 8
        g["offB"], g["wB"] = offB, cB * 8
        offA += g["wA"]
        offB += g["wB"]
    WA, WB = max(offA, 8), max(offB, 8)
    CM = sum(g["tig"] * (g["L0"] + g["L1"]) for g in groups)

    # per-core edge slot assignment (dst side: OLD layout tile/partition)
    dst_owner = node_core[dst]
    loc_row = glob_row[dst] - dst_owner * slice_pad
    # source PHYS row for gather indices
    src_m = glob_row[src] // slice_pad
    src_l = glob_row[src] % slice_pad
    src_row = src_m * slice_pad + old2phys[src_l]

    order = np.lexsort((src_row, ~isA, loc_row, dst_owner))
    so_owner = dst_owner[order]
    so_loc = loc_row[order]
    so_isA = isA[order]
    so_srcrow = src_row[order]
    key = so_owner * (Npad * 2) + so_loc * 2 + (~so_isA).astype(np.int64)
    newrun = np.concatenate([[True], key[1:] != key[:-1]])
    run_start = np.flatnonzero(newrun)
    slot = np.arange(len(key)) - run_start[np.cumsum(newrun) - 1]

    tile_of = so_loc // P
    part_of = so_loc % P
    group_of = np.zeros(n_tiles, np.int64)
    for gi, g in enumerate(groups):
        group_of[g["base"] : g["base"] + g["tig"]] = gi

    idxA_cat, idxB_cat = [], []
    for m in range(ncores):
        emask = so_owner == m
        et = tile_of[emask]
        ep = part_of[emask]
        eA = so_isA[emask]
        esrc = so_srcrow[emask]
        eslot = slot[emask]
        eg = group_of[et]

        iA_full = np.full((16, WA), SENT_A, np.int16)
        iB_full = np.full((16, WB), SENT_B, np.int16)
        for gi, g in enumerate(groups):
            tig, l0, l1 = g["tig"], g["L0"], g["L1"]
            gselA = (eg == gi) & eA
            gselB = (eg == gi) & ~eA
            tt = et - g["base"]
            if l0:
                cidx = tt[gselA] * l0 + eslot[gselA]
                q = cidx * P + ep[gselA]
                iA_full[q % 16, g["offA"] + q // 16] = esrc[gselA].astype(np.int16)
                assert esrc[gselA].max(initial=0) < T0_ROWS
            if l1:
                cidx = tt[gselB] * l1 + eslot[gselB]
                q = cidx * P + ep[gselB]
                iB_full[q % 16, g["offB"] + q // 16] = (
                    esrc[gselB] - T0_ROWS
                ).astype(np.int16)
                assert (esrc[gselB] - T0_ROWS).max(initial=0) < Npad - T0_ROWS
        idxA_cat.append(np.tile(iA_full, (8, 1)))
        idxB_cat.append(np.tile(iB_full, (8, 1)))

    import ml_dtypes

    # per-core xT slice [f_in, slice_pad] (OLD column order), bf16
    xT = []
    for m in range(ncores):
        xs = np.zeros((slice_pad, f_in), np.float32)
        nodes = row_node[m * slice_pad : m * slice_pad + counts[m]]
        xs[: counts[m]] = x[nodes]
        xT.append(np.ascontiguousarray(xs.T).astype(ml_dtypes.bfloat16))

    # replicated full xT [f_in, Npad] (OLD global column order), bf16
    xs_full = np.zeros((Npad, f_in), np.float32)
    valid = row_node >= 0
    xs_full[valid] = x[row_node[valid]]
    xTfull = np.ascontiguousarray(xs_full.T).astype(ml_dtypes.bfloat16)

    # selectors [128, n_tiles*g_per_core] per core (OLD layout)
    sel = []
    for m in range(ncores):
        s = np.zeros((P, n_tiles, g_per_core), np.float32)
        rows = np.arange(counts[m])
        nodes = row_node[m * slice_pad + rows]
        s[rows % P, rows // P, batch[nodes] - m * g_per_core] = 1.0
        sel.append(s.reshape(P, n_tiles * g_per_core))

    struct = dict(
        slice_pad=slice_pad,
        n_tiles=n_tiles,
        Npad=Npad,
        T0_ROWS=T0_ROWS,
        groups=groups,
        WA=WA,
        WB=WB,
        CM=CM,
        f_in=f_in,
        g_per_core=g_per_core,
        ncls=None,
        ncores=ncores,
        ng=ng,
    )
    glob = dict(glob_row=glob_row, row_node=row_node, counts=counts,
                old2phys=old2phys)
    percore = dict(idxA=idxA_cat, idxB=idxB_cat, xT=xT, sel=sel, xTfull=xTfull)
    return struct, percore, glob


def augment_weights(W, a_s, a_d):
    W = np.asarray(W, np.float32)
    a_s = np.asarray(a_s, np.float32)
    a_d = np.asarray(a_d, np.float32)
    As = np.zeros((F_HID, HEADS), np.float32)
    Ad = np.zeros((F_HID, HEADS), np.float32)
    for h in range(HEADS):
        As[h * HID : (h + 1) * HID, h] = a_s[h]
        Ad[h * HID : (h + 1) * HID, h] = a_d[h]
    return np.concatenate([W, W @ As, W @ Ad], axis=1).astype(np.float32)


def make_inmaps2(inputs, struct, percore, layers=3):
    ncores = struct["ncores"]
    ws = [
        augment_weights(inputs[f"W{l}"], inputs[f"as{l}"], inputs[f"ad{l}"])
        for l in range(layers)
    ]
    biases = np.concatenate(
        [np.asarray(inputs[f"b{l}"], np.float32) for l in range(layers)]
    )
    bias_rep = np.tile(biases[None, :], (P, 1))
    wr = np.asarray(inputs["Wr"], np.float32)
    br_rep = np.tile(np.asarray(inputs["br"], np.float32)[None, :], (P, 1))
    in_maps = []
    for m in range(ncores):
        im = dict(
            xT=percore["xT"][m],
            xTfull=percore["xTfull"],
            idxA=percore["idxA"][m],
            idxB=percore["idxB"][m],
            sel=percore["sel"][m],
            biases=bias_rep,
            wr=wr,
            br=br_rep,
        )
        for l in range(layers):
            im[f"w{l}aug"] = ws[l]
        in_maps.append(im)
    return in_maps


def numpy_model2(inputs, struct, percore, glob, layers=3):
    """Numpy re-implementation of the v3 device algorithm."""
    sp = struct["slice_pad"]
    Npad = struct["Npad"]
    T0 = struct["T0_ROWS"]
    ncores = struct["ncores"]
    gpc = struct["g_per_core"]
    nt = struct["n_tiles"]
    o2p = glob["old2phys"]
    ncls = np.asarray(inputs["Wr"]).shape[1]

    ws = [
        augment_weights(inputs[f"W{l}"], inputs[f"as{l}"], inputs[f"ad{l}"])
        for l in range(layers)
    ]
    biases = [np.asarray(inputs[f"b{l}"], np.float32) for l in range(layers)]

    import ml_dtypes

    acts = [None] * ncores
    out_logits = np.zeros((struct["ng"], ncls), np.float32)

    for layer in range(layers):
        table = np.zeros((Npad, 72), np.float32)  # PHYS rows
        for m in range(ncores):
            if layer == 0:
                a = percore["xT"][m].T.astype(np.float32)
            else:
                a = acts[m]
            vals = (a.astype(ml_dtypes.bfloat16).astype(np.float32)
                    @ ws[layer].astype(ml_dtypes.bfloat16).astype(np.float32))
            table[m * sp + o2p, :72] = vals
        # bf16 rounding of h columns (table stored bf16)
        table[:, :64] = table[:, :64].astype(ml_dtypes.bfloat16).astype(np.float32)
        table[T0 - 1, :64] = 0.0
        table[T0 - 1, 64:68] = NEG_BIG
        table[Npad - 1, :64] = 0.0
        table[Npad - 1, 64:68] = NEG_BIG
        for m in range(ncores):
            out = np.zeros((sp, F_HID), np.float32)
            lvals = table[m * sp + o2p, :]  # OLD-layout view of own rows
            asd = lvals[:, 64:72]
            l_self = asd[:, 0:4] + asd[:, 4:8]
            l_self = np.where(l_self >= 0, l_self, NEG_SLOPE * l_self)
            for g in struct["groups"]:
                tig, l0, l1 = g["tig"], g["L0"], g["L1"]
                cA, cB = tig * l0, tig * l1
                C = cA + cB
                iw = percore["idxA"][m][:16, g["offA"] : g["offA"] + g["wA"]]
                iA = iw.T.reshape(-1)[: cA * P].astype(np.int64)
                iw = percore["idxB"][m][:16, g["offB"] : g["offB"] + g["wB"]]
                iB = iw.T.reshape(-1)[: cB * P].astype(np.int64)
                GA = table[:T0][iA].reshape(cA, P, 72).transpose(1, 0, 2)
                GB = table[T0:][iB].reshape(cB, P, 72).transpose(1, 0, 2)
                G = np.concatenate([GA, GB], axis=1)  # [P, C, 72]
                tt = np.concatenate(
                    [np.repeat(np.arange(tig), l0), np.repeat(np.arange(tig), l1)]
                )
                node_rows = (g["base"] + tt)[None, :] * P + np.arange(P)[:, None]
                a_d = lvals[:, 68:72][node_rows]  # [P,C,4]
                logits = G[:, :, 64:68] + a_d
                logits = np.where(logits >= 0, logits, NEG_SLOPE * logits).astype(
                    np.float32
                )
                tiles = g["base"] + np.arange(tig)
                ls_g = l_self[
                    tiles[None, :] * P + np.arange(P)[:, None]
                ]  # [P,tig,4]
                mx = np.full((P, tig, HEADS), NEG_BIG, np.float32)
                np.maximum.at(mx, (slice(None), tt), logits)
                mx = np.maximum(mx, ls_g)
                e = np.exp(logits - mx[:, tt])
                e_self = np.exp(ls_g - mx)
                s = e_self.copy()
                np.add.at(s, (slice(None), tt), e)
                alpha = e * (1.0 / s)[:, tt]
                alpha_self = e_self * (1.0 / s)
                msg = G[:, :, :64].reshape(P, C, HEADS, HID) * alpha[:, :, :, None]
                acc = np.zeros((P, tig, F_HID), np.float32)
                np.add.at(acc, (slice(None), tt), msg.reshape(P, C, F_HID))
                h_self = lvals[:, :64][
                    tiles[None, :] * P + np.arange(P)[:, None]
                ]  # [P,tig,64]
                acc += (
                    h_self.reshape(P, tig, HEADS, HID)
                    * alpha_self[:, :, :, None]
                ).reshape(P, tig, F_HID)
                for t in range(tig):
                    out[(g["base"] + t) * P + np.arange(P)] = acc[:, t]
            act = out + biases[layer][None, :]
            act = act * (1.0 / (1.0 + np.exp(-act)))
            acts[m] = act.astype(np.float32)

    for m in range(ncores):
        sel = percore["sel"][m].reshape(P, nt, gpc)
        a = acts[m].reshape(nt, P, F_HID)
        pooled = np.einsum("ptg,tpf->gf", sel, a)
        lg = pooled @ np.asarray(inputs["Wr"]) + np.asarray(inputs["br"])
        lg = np.maximum(lg, 0.0)
        mxv = lg.max(axis=1, keepdims=True)
        ls = lg - mxv - np.log(np.exp(lg - mxv).sum(axis=1, keepdims=True))
        out_logits[m * gpc : (m + 1) * gpc] = ls
    return out_logits


def build_gat2(S, n_cores=8):
    import os

    dbg_layers = int(os.environ.get("GAT_NLAYERS", "3"))
    dbg_edge = int(os.environ.get("GAT_EDGE", "1"))
    dbg_compute = int(os.environ.get("GAT_COMPUTE", "1"))
    dbg_ngroups = int(os.environ.get("GAT_NGROUPS", "999"))
    dbg_skipcoll = int(os.environ.get("GAT_SKIPCOLL", "0"))
    ELEM = int(os.environ.get("GAT_ELEM", "72"))     # bf16 elems gathered/row
    REPL0 = int(os.environ.get("GAT_REPL0", "0"))    # replicated layer-0 build
    GCH = int(os.environ.get("GAT_GCH", "8"))
    MAXFREE = int(os.environ.get("GAT_MAXFREE", "1"))
    KFLUSH = int(os.environ.get("GAT_KFLUSH", "10"))  # tiles per table flush
    sp = S["slice_pad"]
    nt = S["n_tiles"]
    Npad = S["Npad"]
    T0 = S["T0_ROWS"]
    groups = S["groups"]
    WA = S["WA"]
    WB = S["WB"]
    GPC = S["g_per_core"]
    NCLS = S["ncls"]
    FH = F_HID
    LAYERS = 3
    in_dims = [S["f_in"], FH, FH]

    nc = bacc.Bacc("TRN2", debug=False, num_devices=n_cores, num_swdge_queues=4)

    # ---------------- I/O ----------------
    xT_d = nc.dram_tensor("xT", [in_dims[0], sp], BF16, kind="ExternalInput")
    xTfull_d = nc.dram_tensor("xTfull", [in_dims[0], Npad], BF16,
                              kind="ExternalInput")
    idxA_d = nc.dram_tensor("idxA", [P, WA], I16, kind="ExternalInput")
    idxB_d = nc.dram_tensor("idxB", [P, WB], I16, kind="ExternalInput")
    sel_d = nc.dram_tensor("sel", [P, nt * GPC], F32, kind="ExternalInput")
    w_d = [
        nc.dram_tensor(f"w{l}aug", [in_dims[l], 72], F32, kind="ExternalInput")
        for l in range(LAYERS)
    ]
    bias_d = nc.dram_tensor("biases", [P, LAYERS * FH], F32, kind="ExternalInput")
    wr_d = nc.dram_tensor("wr", [FH, NCLS], F32, kind="ExternalInput")
    br_d = nc.dram_tensor("br", [P, NCLS], F32, kind="ExternalInput")
    out_d = nc.dram_tensor("out", [GPC, NCLS], F32, kind="ExternalOutput")

    table_d = [
        nc.dram_tensor(f"table{l}", [Npad, P], BF16, addr_space="Shared")
        for l in range(LAYERS)
    ]

    rg = [list(range(n_cores))]
    tigmax = max(g["tig"] for g in groups)
    # slices for layers 1.. (layer 0 table is built replicated, no AG)
    slice_d = [
        nc.dram_tensor(f"slice{l}", [sp, P], BF16) for l in range(LAYERS)
    ]

    with tile.TileContext(nc) as tc, ExitStack() as ctx:
        pers = ctx.enter_context(tc.tile_pool(name="pers", bufs=1))
        gpool = ctx.enter_context(
            tc.tile_pool(name="G", bufs=int(os.environ.get("GAT_GBUFS", "5")))
        )
        ltpool = ctx.enter_context(tc.tile_pool(name="lt", bufs=3))
        xpool = ctx.enter_context(tc.tile_pool(name="xchunk", bufs=4))
        stat = ctx.enter_context(tc.tile_pool(name="stat", bufs=3))
        opool = ctx.enter_context(tc.tile_pool(name="oacc", bufs=3))
        rowp = ctx.enter_context(tc.tile_pool(name="row", bufs=3))
        psum = ctx.enter_context(tc.tile_pool(name="psum", bufs=2, space="PSUM"))
        psumT = ctx.enter_context(tc.tile_pool(name="psumT", bufs=2, space="PSUM"))

        # ---- persistent SBUF ----
        sel_sb = pers.tile([P, nt * GPC], F32)
        w_sb = [
            pers.tile([in_dims[l], 72], F32, name=f"w{l}sb", tag=f"w{l}sb")
            for l in range(LAYERS)
        ]
        wb_sb = [
            pers.tile([in_dims[l], 72], BF16, name=f"wb{l}sb", tag=f"wb{l}sb")
            for l in range(LAYERS)
        ]
        bias_sb = pers.tile([P, LAYERS * FH], F32)
        wr_sb = pers.tile([FH, NCLS], F32)
        br_sb = pers.tile([P, NCLS], F32)
        idxA_sb = pers.tile([P, WA], I16)
        idxB_sb = pers.tile([P, WB], I16)
        # rows staged at 256B pitch so table/slice flushes are contiguous
        rows_sb = pers.tile([P, nt * P], BF16)    # [h(64)|a_src raw(8)|junk]
        asd_sb = pers.tile([P, nt * 8], F32)      # [a_src(4)|a_dst(4)] per tile
        lself_sb = pers.tile([P, nt * HEADS], F32)  # leaky(a_s+a_d) per tile
        out_sb = pers.tile([P, nt * FH], F32)
        actT_sb = pers.tile([FH, sp], BF16)
        xT_sb = pers.tile([in_dims[0], sp], BF16)
        nc.sync.dma_start(xT_sb[:], xT_d[:])
        ident = pers.tile([P, P], F32)

        nc.sync.dma_start(sel_sb[:], sel_d[:])
        for l in range(LAYERS):
            nc.sync.dma_start(w_sb[l][:], w_d[l][:])
        nc.sync.dma_start(bias_sb[:], bias_d[:])
        nc.sync.dma_start(wr_sb[:], wr_d[:])
        nc.sync.dma_start(br_sb[:], br_d[:])
        nc.sync.dma_start(idxA_sb[:], idxA_d[:])
        nc.sync.dma_start(idxB_sb[:], idxB_d[:])
        for l in range(LAYERS):
            nc.vector.tensor_copy(wb_sb[l][:], w_sb[l][:])
        make_identity(nc, ident[:])

        nc.gpsimd.load_library(mlp_lib)

        _regs = {}
        qrr = [0]

        def nreg(v):
            if v not in _regs:
                _regs[v] = nc.gpsimd.to_reg(v)
            return _regs[v]

        rows3 = rows_sb[:].rearrange("p (t f) -> p t f", f=P)
        asd3 = asd_sb[:].rearrange("p (t x) -> p t x", x=8)
        lself3 = lself_sb[:].rearrange("p (t h) -> p t h", h=HEADS)

        # sentinel row constant: h=0, a_src=-1e30 (raw f32 bits)
        sent_sb = pers.tile([1, 72], BF16, tag="sent")
        nc.vector.memset(sent_sb[:, :64], 0.0)
        nc.vector.memset(sent_sb[:, 64:72].bitcast(F32), NEG_BIG)

        # PHYS view of a table/slice: rows (p nt + t) -> [p, t, elem]
        def phys_rows(dram, lo_row, n_rows_tiles):
            return dram[lo_row : lo_row + P * n_rows_tiles, :].rearrange(
                "(p t) e -> p t e", t=n_rows_tiles
            )

        def build_table0_repl():
            """Every core writes the FULL layer-0 table from xTfull."""
            for m in range(n_cores):
                t = 0
                while t < nt:
                    k = min(KFLUSH, nt - t)
                    st = rowp.tile([P, KFLUSH * P], BF16, tag="t0stage")
                    for j in range(k):
                        xc = xpool.tile([in_dims[0], P], BF16, tag="xfc")
                        col = m * sp + (t + j) * P
                        nc.sync.dma_start(xc[:], xTfull_d[:, col : col + P])
                        pR = psum.tile([P, 72], F32)
                        nc.tensor.matmul(
                            pR[:], lhsT=xc[:], rhs=wb_sb[0][:], start=True, stop=True
                        )
                        sl = st[:].rearrange("p (j f) -> p j f", f=P)
                        nc.scalar.copy(sl[:, j, :64], pR[:, :64])
                        nc.vector.tensor_copy(
                            sl[:, j, 64:72].bitcast(F32), pR[:, 64:68]
                        )
                    # flush k tiles: phys rows m*sp + p*nt + (t..t+k)
                    nc.sync.dma_start(
                        phys_rows(table_d[0], m * sp, nt)[:, t : t + k, :],
                        st[:].rearrange("p (j f) -> p j f", f=P)[:, :k, :],
                    )
                    t += k
            # sentinel rows (phys T0-1 and Npad-1)
            nc.sync.dma_start(table_d[0][T0 - 1 : T0, :72], sent_sb[:])
            nc.sync.dma_start(table_d[0][Npad - 1 : Npad, :72], sent_sb[:])

        def build_slice_tile(layer, c, write_slice=True):
            """Matmul tile c of layer's rows into rows_sb + asd_sb (+ lself).
            For layer 0 lhsT comes from xT (local extras only)."""
            if layer == 0:
                lhsT_ap = xT_sb[:, c * P : (c + 1) * P]
            else:
                pT = psumT.tile([FH, P], F32)
                nc.tensor.transpose(
                    out=pT[:],
                    in_=out_sb[:, c * FH : (c + 1) * FH],
                    identity=ident[:],
                )
                nc.scalar.copy(actT_sb[:, c * P : (c + 1) * P], pT[:])
                lhsT_ap = actT_sb[:, c * P : (c + 1) * P]
            pR = psum.tile([P, 72], F32)
            nc.tensor.matmul(pR[:], lhsT=lhsT_ap, rhs=wb_sb[layer][:], start=True,
                             stop=True)
            nc.scalar.copy(rows3[:, c, :64], pR[:, :64])  # h -> bf16
            nc.vector.tensor_copy(rows3[:, c, 64:72].bitcast(F32), pR[:, 64:68])
            nc.vector.tensor_copy(asd3[:, c, :], pR[:, 64:72])
            # l_self = leaky(a_src + a_dst)
            ls = lself3[:, c, :]
            nc.vector.tensor_tensor(out=ls, in0=asd3[:, c, 0:4],
                                    in1=asd3[:, c, 4:8], op=OP.add)
            lst = stat.tile([P, HEADS], F32, tag="lst")
            nc.vector.tensor_scalar_mul(lst[:], ls, NEG_SLOPE)
            nc.vector.tensor_tensor(out=ls, in0=ls, in1=lst[:], op=OP.max)


        def flush_slice(layer, t0, t1):
            """Write rows_sb tiles [t0,t1) into slice_d[layer] (phys rows)."""
            nc.sync.dma_start(
                phys_rows(slice_d[layer], 0, nt)[:, t0:t1, :],
                rows3[:, t0:t1, :],
            )

        def do_allgather(layer):
            if dbg_skipcoll:
                return
            nc.gpsimd.collective_compute(
                "AllGather",
                mybir.AluOpType.bypass,
                replica_groups=rg,
                ins=[slice_d[layer].ap().opt()],
                outs=[table_d[layer].ap().opt()],
            )

        def edge_group(layer, g):
            tig, l0, l1 = g["tig"], g["L0"], g["L1"]
            cA, cB = tig * l0, tig * l1
            C = cA + cB
            base = g["base"]

            if C == 0:
                osl = out_sb[:, base * FH : (base + tig) * FH]
                nc.vector.tensor_copy(
                    osl.rearrange("p (t f) -> p t f", f=FH),
                    rows3[:, base : base + tig, :64],
                )
                return

            G = gpool.tile([P, C * ELEM], BF16, tag="G")
            G3 = G[:].rearrange("p (c f) -> p c f", f=ELEM)
            for c0all, ccn, itile, ioff, tdsl in (
                (0, cA, idxA_sb, g["offA"], table_d[layer][:T0, :ELEM]),
                (cA, cB, idxB_sb, g["offB"], table_d[layer][T0:, :ELEM]),
            ):
                if ccn == 0:
                    continue
                for k in range(0, ccn, GCH):
                    kc = min(GCH, ccn - k)
                    dma_gather_narrow(
                        nc.gpsimd,
                        G3[:, c0all + k : c0all + k + kc, :],
                        tdsl,
                        itile[:, (ioff + k * 8) : (ioff + (k + kc) * 8)],
                        kc * P,
                        nreg(kc * P),
                        ELEM,
                        P,
                        queue_num=qrr[0] % 4,
                    )
                    qrr[0] += 1

            if not dbg_compute:
                nc.vector.tensor_copy(
                    out_sb[:, base * FH : (base + tig) * FH].rearrange(
                        "p (t f) -> p t f", f=64
                    ),
                    G3[:, :tig, :64],
                )
                return
            lt = ltpool.tile([P, C * HEADS], F32, tag="lt")
            lt3 = lt[:].rearrange("p (c h) -> p c h", h=HEADS)
            # logits = a_src[gathered] + a_dst[local]
            for c0, cc, L in ((0, cA, l0), (cA, cB, l1)):
                if cc == 0:
                    continue
                adview = (
                    asd3[:, base : base + tig, 4:8]
                    .unsqueeze(2)
                    .broadcast_to([P, tig, L, HEADS])
                )
                nc.vector.tensor_tensor(
                    out=lt3[:, c0 : c0 + cc, :].rearrange(
                        "p (t l) h -> p t l h", l=L
                    ),
                    in0=G3[:, c0 : c0 + cc, 64:72]
                    .bitcast(F32)
                    .rearrange("p (t l) h -> p t l h", l=L),
                    in1=adview,
                    op=OP.add,
                )
            # leaky relu
            lt2 = ltpool.tile([P, C * HEADS], F32, tag="lt2")
            nc.vector.tensor_scalar_mul(lt2[:], lt[:], NEG_SLOPE)
            nc.vector.tensor_tensor(out=lt[:], in0=lt[:], in1=lt2[:], op=OP.max)
            es = stat.tile([P, tigmax * HEADS], F32, tag="es")
            if MAXFREE:
                # logits are small (|l| < ~20): skip the max-subtraction pass
                nc.scalar.activation(lt[:], lt[:], AF.Exp)
                nc.scalar.activation(
                    es[:, : tig * HEADS],
                    lself_sb[:, base * HEADS : (base + tig) * HEADS],
                    AF.Exp,
                )
            else:
                # segment max over slots + self
                m1 = stat.tile([P, tigmax * HEADS], F32, tag="m1")
                if cA:
                    nc.vector.reduce_max(
                        m1[:].rearrange("p (t h) -> p t h", h=HEADS)[:, :tig, :],
                        lt3[:, :cA, :].rearrange("p (t l) h -> p t h l", l=l0),
                        axis=mybir.AxisListType.X,
                    )
                if cB:
                    m2 = stat.tile([P, tigmax * HEADS], F32, tag="m2")
                    nc.vector.reduce_max(
                        m2[:].rearrange("p (t h) -> p t h", h=HEADS)[:, :tig, :],
                        lt3[:, cA:, :].rearrange("p (t l) h -> p t h l", l=l1),
                        axis=mybir.AxisListType.X,
                    )
                    nc.vector.tensor_tensor(
                        out=m1[:, : tig * HEADS],
                        in0=m1[:, : tig * HEADS] if cA else m2[:, : tig * HEADS],
                        in1=m2[:, : tig * HEADS],
                        op=OP.max,
                    )
                nc.vector.tensor_tensor(
                    out=m1[:, : tig * HEADS],
                    in0=m1[:, : tig * HEADS],
                    in1=lself_sb[:, base * HEADS : (base + tig) * HEADS],
                    op=OP.max,
                )
                # e = exp(l - m)
                for c0, cc, L in ((0, cA, l0), (cA, cB, l1)):
                    if cc == 0:
                        continue
                    mview = (
                        m1[:, : tig * HEADS]
                        .rearrange("p (t h) -> p t h", h=HEADS)
                        .unsqueeze(2)
                        .broadcast_to([P, tig, L, HEADS])
                    )
                    ltv = lt3[:, c0 : c0 + cc, :].rearrange(
                        "p (t l) h -> p t l h", l=L
                    )
                    nc.vector.tensor_tensor(
                        out=ltv, in0=ltv, in1=mview, op=OP.subtract
                    )
                nc.scalar.activation(lt[:], lt[:], AF.Exp)
                # e_self = exp(l_self - m)
                nc.vector.tensor_tensor(
                    out=es[:, : tig * HEADS],
                    in0=lself_sb[:, base * HEADS : (base + tig) * HEADS],
                    in1=m1[:, : tig * HEADS],
                    op=OP.subtract,
                )
                nc.scalar.activation(
                    es[:, : tig * HEADS], es[:, : tig * HEADS], AF.Exp
                )
            # s = sum(e) + e_self ; r = 1/s
            s1 = stat.tile([P, tigmax * HEADS], F32, tag="s1")
            if cA:
                nc.vector.reduce_sum(
                    s1[:].rearrange("p (t h) -> p t h", h=HEADS)[:, :tig, :],
                    lt3[:, :cA, :].rearrange("p (t l) h -> p t h l", l=l0),
                    axis=mybir.AxisListType.X,
                )
            if cB:
                s2 = stat.tile([P, tigmax * HEADS], F32, tag="s2")
                nc.vector.reduce_sum(
                    s2[:].rearrange("p (t h) -> p t h", h=HEADS)[:, :tig, :],
                    lt3[:, cA:, :].rearrange("p (t l) h -> p t h l", l=l1),
                    axis=mybir.AxisListType.X,
                )
                nc.vector.tensor_tensor(
                    out=s1[:, : tig * HEADS],
                    in0=s1[:, : tig * HEADS] if cA else s2[:, : tig * HEADS],
                    in1=s2[:, : tig * HEADS],
                    op=OP.add,
                )
            nc.vector.tensor_tensor(
                out=s1[:, : tig * HEADS],
                in0=s1[:, : tig * HEADS],
                in1=es[:, : tig * HEADS],
                op=OP.add,
            )
            nc.vector.reciprocal(s1[:, : tig * HEADS], s1[:, : tig * HEADS])
            # alpha = e * r ; alpha_self = e_self * r
            ab = ltpool.tile([P, C * HEADS], BF16, tag="ab")
            ab3 = ab[:].rearrange("p (c h) -> p c h", h=HEADS)
            for c0, cc, L in ((0, cA, l0), (cA, cB, l1)):
                if cc == 0:
                    continue
                rview = (
                    s1[:, : tig * HEADS]
                    .rearrange("p (t h) -> p t h", h=HEADS)
                    .unsqueeze(2)
                    .broadcast_to([P, tig, L, HEADS])
                )
                ltv = lt3[:, c0 : c0 + cc, :].rearrange("p (t l) h -> p t l h", l=L)
                abv = ab3[:, c0 : c0 + cc, :].rearrange("p (t l) h -> p t l h", l=L)
                nc.vector.tensor_tensor(out=abv, in0=ltv, in1=rview, op=OP.mult)
            nc.vector.tensor_tensor(
                out=es[:, : tig * HEADS],
                in0=es[:, : tig * HEADS],
                in1=s1[:, : tig * HEADS],
                op=OP.mult,
            )
            # msg = h * alpha (in place on G, bf16), 4D broadcast over HID
            gv = G3[:, :C, :64].rearrange("p c (h d) -> p c h d", d=HID)
            av = (
                ab3[:, :C, :]
                .unsqueeze(3)
                .broadcast_to([P, C, HEADS, HID])
            )
            nc.vector.tensor_tensor(out=gv, in0=gv, in1=av, op=OP.mult)
            # out = segment sum of messages (+ self)
            oA = opool.tile([P, tigmax * FH], F32, tag="oA")
            oB = opool.tile([P, tigmax * FH], F32, tag="oB")
            if cA:
                nc.vector.reduce_sum(
                    oA[:].rearrange("p (t f) -> p t f", f=FH)[:, :tig, :],
                    G3[:, :cA, :64].rearrange("p (t l) f -> p t f l", l=l0),
                    axis=mybir.AxisListType.X,
                )
            if cB:
                nc.vector.reduce_sum(
                    oB[:].rearrange("p (t f) -> p t f", f=FH)[:, :tig, :],
                    G3[:, cA:, :64].rearrange("p (t l) f -> p t f l", l=l1),
                    axis=mybir.AxisListType.X,
                )
            osl = out_sb[:, base * FH : (base + tig) * FH]
            if cA and cB:
                nc.vector.tensor_tensor(
                    out=osl, in0=oA[:, : tig * FH], in1=oB[:, : tig * FH], op=OP.add
                )
            else:
                nc.vector.tensor_copy(
                    osl, (oA if cA else oB)[:, : tig * FH]
                )
            # self message: osl += h_local * alpha_self
            stmp = opool.tile([P, tigmax * FH], F32, tag="stmp")
            nc.vector.tensor_tensor(
                out=stmp[:, : tig * FH].rearrange(
                    "p (t h d) -> p t h d", h=HEADS, d=HID
                ),
                in0=rows3[:, base : base + tig, :64].rearrange(
                    "p t (h d) -> p t h d", d=HID
                ),
                in1=es[:, : tig * HEADS]
                .rearrange("p (t h) -> p t h", h=HEADS)
                .unsqueeze(3)
                .broadcast_to([P, tig, HEADS, HID]),
                op=OP.mult,
            )
            nc.vector.tensor_tensor(
                out=osl, in0=osl, in1=stmp[:, : tig * FH], op=OP.add
            )
            # bias + silu
            blg = (
                bias_sb[:, layer * FH : (layer + 1) * FH]
                .unsqueeze(1)
                .broadcast_to([P, tig, FH])
            )
            nc.vector.tensor_tensor(
                out=osl.rearrange("p (t f) -> p t f", f=FH),
                in0=osl.rearrange("p (t f) -> p t f", f=FH),
                in1=blg,
                op=OP.add,
            )
            sgg = stat.tile([P, tigmax * FH], F32, tag="sgg")
            nc.scalar.activation(sgg[:, : tig * FH], osl, AF.Sigmoid)
            nc.vector.tensor_tensor(
                out=osl, in0=osl, in1=sgg[:, : tig * FH], op=OP.mult
            )

        # ============ layer 0 table + local extras ============
        if REPL0:
            build_table0_repl()
            for c in range(nt):
                build_slice_tile(0, c, write_slice=False)
        else:
            for c in range(nt):
                build_slice_tile(0, c)
            flush_slice(0, 0, nt)
            # sentinel at phys slice row sp-1 (overwrites junk row)
            nc.sync.dma_start(slice_d[0][sp - 1 : sp, :72], sent_sb[:])
            do_allgather(0)

        # ============ layers ============
        pP = psum.tile([GPC, FH], F32, tag="pool", bufs=1)
        pool_done = 0  # tiles accumulated into the pooling PSUM so far

        def pool_tiles(hi):
            nonlocal pool_done
            for t in range(pool_done, hi):
                nc.tensor.matmul(
                    pP[:],
                    lhsT=sel_sb[:, t * GPC : (t + 1) * GPC],
                    rhs=out_sb[:, t * FH : (t + 1) * FH],
                    start=(t == 0),
                    stop=(t == nt - 1),
                )
            pool_done = hi

        for layer in range(dbg_layers):
            built = 0
            flushed = 0
            last = layer == dbg_layers - 1
            for gi, g in enumerate(groups[:dbg_ngroups] if dbg_edge else []):
                edge_group(layer, g)
                nxt = layer + 1
                if nxt < dbg_layers:
                    for c in range(built, g["base"] + g["tig"]):
                        build_slice_tile(nxt, c)
                    built = g["base"] + g["tig"]
                    if built - flushed >= KFLUSH:
                        flush_slice(nxt, flushed, built)
                        flushed = built
                elif last and dbg_edge and dbg_ngroups >= len(groups):
                    pool_tiles(g["base"] + g["tig"])
            nxt = layer + 1
            if nxt < dbg_layers:
                for c in range(built, nt):
                    build_slice_tile(nxt, c)
                flush_slice(nxt, flushed, nt)
                # sentinel at phys slice row sp-1 (overwrites junk row)
                nc.sync.dma_start(slice_d[nxt][sp - 1 : sp, :72], sent_sb[:])
                do_allgather(nxt)

        # ============ pooling + classifier ============
        pool_tiles(nt)
        pooled = rowp.tile([GPC, FH], F32, tag="pooled")
        nc.vector.tensor_copy(pooled[:], pP[:])
        pTpsum = psumT.tile([FH, GPC], F32, tag="poolT", bufs=1)
        nc.tensor.transpose(out=pTpsum[:], in_=pooled[:], identity=ident[:GPC, :GPC])
        pooledT = rowp.tile([FH, GPC], F32, tag="pooledT")
        nc.vector.tensor_copy(pooledT[:], pTpsum[:])
        lgP = psum.tile([GPC, NCLS], F32, tag="lg", bufs=1)
        nc.tensor.matmul(lgP[:], lhsT=pooledT[:], rhs=wr_sb[:], start=True, stop=True)
        lg = rowp.tile([GPC, NCLS], F32, tag="lgs")
        nc.vector.tensor_tensor(out=lg[:], in0=lgP[:], in1=br_sb[:GPC, :], op=OP.add)
        nc.scalar.activation(lg[:], lg[:], AF.Relu)
        mx = stat.tile([GPC, 1], F32, tag="mx")
        nc.vector.reduce_max(mx[:], lg[:], axis=mybir.AxisListType.X)
        nc.vector.tensor_tensor(
            out=lg[:], in0=lg[:], in1=mx[:].broadcast_to([GPC, NCLS]), op=OP.subtract
        )
        ex = rowp.tile([GPC, NCLS], F32, tag="ex")
        nc.scalar.activation(ex[:], lg[:], AF.Exp)
        sm = stat.tile([GPC, 1], F32, tag="sm")
        nc.vector.reduce_sum(sm[:], ex[:], axis=mybir.AxisListType.X)
        lnm = stat.tile([GPC, 1], F32, tag="lnm")
        nc.scalar.activation(lnm[:], sm[:], AF.Ln)
        nc.vector.tensor_tensor(
            out=lg[:], in0=lg[:], in1=lnm[:].broadcast_to([GPC, NCLS]), op=OP.subtract
        )
        nc.sync.dma_start(out_d[:], lg[:])

    nc.compile()
    return nc


# ======== runner ========
"""kernel(**inputs) entry point: shard, build, run on 8 cores, gather."""

import os as _os

_NCORES = 8
_NG = 128


def kernel(**inputs) -> "np.ndarray":
    x = np.asarray(inputs["x"], np.float32)
    ei = np.asarray(inputs["edge_index"])
    batch = np.asarray(inputs["batch"])

    struct, percore, glob = build_plan2(
        x, ei, batch, ng=_NG, ncores=_NCORES,
        ccap=int(_os.environ.get("GAT_CCAP", "64")),
    )
    struct["ncls"] = int(np.asarray(inputs["Wr"]).shape[1])

    nc = build_gat2(struct, n_cores=_NCORES)
    in_maps = make_inmaps2(inputs, struct, percore)

    from concourse.bass_utils import run_bass_kernel_spmd

    trace = _os.environ.get("GAT_TRACE", "0") == "1"
    res = run_bass_kernel_spmd(
        nc,
        in_maps,
        core_ids=list(range(_NCORES)),
        trace=trace,
    )
    if res.exec_time_ns is not None:
        print(f"HW exec time: {res.exec_time_ns} ns", flush=True)

    out = np.concatenate([res.results[m]["out"] for m in range(_NCORES)], axis=0)
    return out.astype(np.float32)
